# revision 37
# baseline (speedup 1.0000x reference)
"""Trainium2 Bass kernel for nn_AdaAug (scatter_memory).

Computation (per sample i, kriged node k):
    r          = offs[i] + krig_idx[i,k]            # flat row index
    smp        = y[r, :]                            # gather
    h          = relu(smp @ W1 + b1)
    logits     = h @ W2 + b2
    ind        = argmax(logits + gumbel) == 1       # hard gumbel-softmax fwd
    out        = x, with out[r, :] = ind * mask * smp

Sharding: data-parallel over batch across 8 NeuronCores (64 samples per
core); MLP weights replicated; gathers/scatters are device-local because
krig_idx offsets stay within each sample's 500-row block.

Device-side structure per core:
  - 7 InstDMAGatherAnt pieces gather the 6400 kriged y rows (512B padded
    rows) into SBUF [128, blocks, 128] tiles; gathered pair j lands at
    partition j%128, block j//128; no piece straddles the output-half
    boundary so each scatter's dependencies resolve as early as possible
  - tiny MLP on TensorE/ScalarE in 4-block batches (4 PE transposes ->
    one DVE PSUM copy -> one W1 matmul -> one relu -> 4 W2 matmuls),
    indicator via one DVE is_gt per piece
  - bulk x->out copy as 2 strided DRAM->DRAM chunk DMAs into the padded
    (row stride 512B) output halves; the kriged rows of x are pre-zeroed
    host-side so the scatter can be a CCE add
  - 2 InstDMAScatterAddAnt instructions land the 3200 val rows per half.
    (The InstDMACopy indirect-scatter path costs ~2.2us of serial GpSimd
    time per 128 rows; the custom scatter-add retires at descriptor
    generation, ~8ns/row.)

Host side does only sharding/layout prep: flat index computation, int16
wrapping, row padding of y, kriged-row zeroing of x, un-padding of the
outputs.
"""

import sys

import numpy as np

for _p in ("/opt/trn_rl_repo", "/opt/pypackages"):
    if _p not in sys.path:
        sys.path.insert(0, _p)

M = 8                 # cores
BS, N, K, S = 512, 500, 100, 96
HID, AUG = 32, 2
B = BS // M           # samples per core
R = B * N             # x/y rows per core
J = B * K             # gathered rows per core
P = 128               # SBUF partitions
T = J // P            # blocks of 128 gathered rows
C = 2                 # output chunks per core (3200 pairs = 25 blocks each)
RC = R // C           # rows per output chunk
JC = J // C           # gathered rows per output chunk
TC = T // C           # blocks per output chunk
SP = 128              # padded row width (512B)
PIECES = (5, 5, 10, 5, 5, 10, 10)   # gather pieces; none straddles block TC=25
NG = len(PIECES)
POFF = [sum(PIECES[:i]) for i in range(NG + 1)]  # block offsets

_cache = {}


def _build():
    from contextlib import ExitStack

    import concourse.tile as tile
    from concourse import bacc, library_config, mybir

    f32 = mybir.dt.float32
    i16 = mybir.dt.int16

    nc = bacc.Bacc("TRN2", target_bir_lowering=False, debug=False, num_devices=M)

    x_e = nc.dram_tensor("x", [R, S], f32, kind="ExternalInput")
    yp_e = nc.dram_tensor("yp", [R, SP], f32, kind="ExternalInput")
    mask_e = nc.dram_tensor("mask", [P, T * S], f32, kind="ExternalInput")
    gum_e = nc.dram_tensor("gum", [P, T * AUG], f32, kind="ExternalInput")
    w1_e = nc.dram_tensor("W1", [S, HID], f32, kind="ExternalInput")
    b1_e = nc.dram_tensor("b1", [HID, 1], f32, kind="ExternalInput")
    w2_e = nc.dram_tensor("W2", [HID, AUG], f32, kind="ExternalInput")
    b2_e = nc.dram_tensor("b2", [P, AUG], f32, kind="ExternalInput")
    ident_e = nc.dram_tensor("ident", [P, P], f32, kind="ExternalInput")
    gidx_e = nc.dram_tensor("gidx", [P, J // 16], i16, kind="ExternalInput")
    sidx_e = nc.dram_tensor("sidx", [P, J // 16], i16, kind="ExternalInput")
    outs = [
        nc.dram_tensor(f"out{g}", [RC, SP], f32, kind="ExternalOutput")
        for g in range(C)
    ]

    with tile.TileContext(nc) as tc, ExitStack() as ctx:
        const = ctx.enter_context(tc.tile_pool(name="const", bufs=1))
        big = ctx.enter_context(tc.tile_pool(name="big", bufs=1))
        work = ctx.enter_context(tc.tile_pool(name="work", bufs=3))
        pp = ctx.enter_context(tc.tile_pool(name="pp", bufs=2, space="PSUM"))
        ppl = ctx.enter_context(tc.tile_pool(name="ppl", bufs=1, space="PSUM"))

        nc.gpsimd.load_library(library_config.mlp)

        gidx_sb = const.tile([P, J // 16], i16)
        nc.sync.dma_start(gidx_sb[:], gidx_e[:])
        sidx_sb = const.tile([P, J // 16], i16)
        nc.sync.dma_start(sidx_sb[:], sidx_e[:])
        w1_sb = const.tile([S, HID], f32)
        nc.sync.dma_start(w1_sb[:], w1_e[:])
        b1_sb = const.tile([HID, 1], f32)
        nc.sync.dma_start(b1_sb[:], b1_e[:])
        w2_sb = const.tile([HID, AUG], f32)
        nc.sync.dma_start(w2_sb[:], w2_e[:])
        b2_sb = const.tile([P, AUG], f32)
        nc.sync.dma_start(b2_sb[:], b2_e[:])
        gum_sb = const.tile([P, T * AUG], f32)
        nc.sync.dma_start(gum_sb[:], gum_e[:])
        ident = const.tile([P, P], f32)
        nc.sync.dma_start(ident[:], ident_e[:])

        mask_g = big.tile([P, T * S], f32)
        nc.sync.dma_start(mask_g[:, : TC * S], mask_e[:][:, : TC * S])

        # Gather the kriged y rows in NG pieces (per-piece tiles keep the
        # dependencies piece-local so the MLP starts after piece 0).
        smp_gs = [
            big.tile([P, PIECES[gp] * SP], f32, name=f"smp{gp}", tag=f"smp{gp}")
            for gp in range(NG)
        ]
        for gp in range(NG):
            jg = PIECES[gp] * P
            c0 = POFF[gp] * P // 16
            nc.gpsimd.dma_gather(
                out_ap=smp_gs[gp][:].rearrange("p (t e) -> p t e", e=SP),
                in_ap=yp_e[:],
                idxs_ap=gidx_sb[:, c0 : c0 + jg // 16],
                num_idxs=jg,
                num_idxs_reg=jg,
                elem_size=SP,
                single_packet=False,
            )

        # Bulk copy x into the padded output halves (x has the kriged rows
        # pre-zeroed, so the later scatter-add writes val onto zeros).
        # copy0 issues from the sync HWDGE; copy1 from the gpsimd SWDGE
        # (emitted between gather pieces below) so the two 16000-descriptor
        # strided issues don't serialize on one descriptor generator.
        nc.sync.dma_start(outs[0][:][:, 0:S], x_e[0:RC, :])
        nc.sync.dma_start(mask_g[:, TC * S :], mask_e[:][:, TC * S :])
        nc.sync.dma_start(outs[1][:][:, 0:S], x_e[RC : 2 * RC, :])

        # val halves, written per 5-block half-aligned subgroup so each
        # scatter-add's dependencies stay precise.
        val_hs = [
            big.tile([P, TC * S], f32, name=f"valh{g}", tag=f"valh{g}")
            for g in range(C)
        ]
        thr = const.tile([P, 1], f32)
        nc.vector.tensor_tensor(
            out=thr[:],
            in0=b2_sb[:, 0:1],
            in1=b2_sb[:, 1:2],
            op=mybir.AluOpType.subtract,
        )
        g3 = gum_sb[:].rearrange("p (t a) -> p t a", a=AUG)

        for gp in range(NG):
            smp_g = smp_gs[gp]
            TGp = PIECES[gp]
            logits_p = ppl.tile([P, TGp * AUG], f32, name=f"lp{gp}", tag="lp", bufs=2)
            # MLP in 4-block batches: per batch, 4 PE transposes into one
            # PSUM tile, one DVE copy, one W1 matmul, one relu, 4 W2 matmuls.
            for b0 in range(0, TGp, 4):
                bl = min(4, TGp - b0)
                smp_tp = pp.tile(
                    [S, 4 * P], f32, name=f"smp_tp{gp}_{b0}", tag="smp_tp"
                )
                for b in range(bl):
                    nc.tensor.transpose(
                        smp_tp[:, b * P : (b + 1) * P],
                        smp_g[:, (b0 + b) * SP : (b0 + b) * SP + S],
                        ident[:],
                    )
                smp_ts = work.tile(
                    [S, 4 * P], f32, name=f"smp_ts{gp}_{b0}", tag="smp_ts"
                )
                nc.vector.tensor_copy(
                    smp_ts[:, : bl * P], smp_tp[:, : bl * P]
                )
                h_p = pp.tile([HID, 4 * P], f32, name=f"h_p{gp}_{b0}", tag="h_p")
                nc.tensor.matmul(
                    h_p[:, : bl * P],
                    lhsT=w1_sb[:],
                    rhs=smp_ts[:, : bl * P],
                    start=True,
                    stop=True,
                )
                h_s = work.tile([HID, 4 * P], f32, name=f"h_s{gp}_{b0}", tag="h_s")
                nc.scalar.activation(
                    h_s[:, : bl * P],
                    h_p[:, : bl * P],
                    mybir.ActivationFunctionType.Relu,
                    bias=b1_sb[:],
                )
                for b in range(bl):
                    nc.tensor.matmul(
                        logits_p[:, (b0 + b) * AUG : (b0 + b + 1) * AUG],
                        lhsT=h_s[:, b * P : (b + 1) * P],
                        rhs=w2_sb[:],
                        start=True,
                        stop=True,
                    )

            # indicator = (l1 + g1 + b2[1] > l0 + g0 + b2[0]) for this piece
            tsl = slice(POFF[gp], POFF[gp + 1])
            logits_sb = work.tile([P, TGp * AUG], f32, name=f"lsb{gp}", tag="lsb")
            nc.vector.tensor_copy(logits_sb[:], logits_p[:])
            l3 = logits_sb[:].rearrange("p (t a) -> p t a", a=AUG)
            ld = work.tile([P, TGp], f32, name=f"ld{gp}", tag="ld")
            nc.vector.tensor_tensor(
                out=ld[:], in0=l3[:, :, 1], in1=l3[:, :, 0],
                op=mybir.AluOpType.subtract,
            )
            e_sb = work.tile([P, TGp], f32, name=f"e{gp}", tag="e")
            nc.vector.tensor_tensor(
                out=e_sb[:], in0=g3[:, tsl, 1], in1=g3[:, tsl, 0],
                op=mybir.AluOpType.subtract,
            )
            nc.vector.tensor_tensor(
                out=e_sb[:], in0=e_sb[:], in1=ld[:], op=mybir.AluOpType.add
            )
            ind = work.tile([P, TGp], f32, name=f"ind{gp}", tag="ind")
            nc.vector.tensor_tensor(
                out=ind[:],
                in0=e_sb[:],
                in1=thr[:].to_broadcast([P, TGp]),
                op=mybir.AluOpType.is_gt,
            )

            # val = ind * mask * smp (whole piece lies within one output half)
            t0 = POFF[gp]
            g_out = t0 // TC
            o0 = (t0 % TC) * S
            vslice = val_hs[g_out][:, o0 : o0 + TGp * S]
            v3 = vslice.rearrange("p (t s) -> p t s", s=S)
            nc.vector.tensor_tensor(
                out=v3,
                in0=smp_g[:].rearrange("p (t e) -> p t e", e=SP)[:, :, 0:S],
                in1=mask_g[:, t0 * S : (t0 + TGp) * S].rearrange(
                    "p (t s) -> p t s", s=S
                ),
                op=mybir.AluOpType.mult,
            )
            ind_b = ind[:].unsqueeze(2).to_broadcast([P, TGp, S])
            nc.vector.tensor_tensor(
                out=v3, in0=v3, in1=ind_b, op=mybir.AluOpType.mult
            )

        # Scatter-add the val rows onto the pre-zeroed kriged rows.
        for g in range(C):
            nc.gpsimd.dma_scatter_add(
                out_ap=outs[g][:][:, 0:S],
                in_ap=val_hs[g][:].rearrange("p (t s) -> p t s", s=S),
                idxs_ap=sidx_sb[:, g * (JC // 16) : (g + 1) * (JC // 16)],
                num_idxs=JC,
                num_idxs_reg=JC,
                elem_size=S,
                elem_step=SP,
                single_packet=False,
            )

    nc.compile()
    return nc


def _get_nc():
    if "nc" not in _cache:
        _cache["nc"] = _build()
    return _cache["nc"]


def _numpy_fallback(x, y, W1, b1, W2, b2, mask, gumbel, krig_idx, idx_of_node):
    offs = np.concatenate([[0], np.cumsum(idx_of_node.astype(np.int64))[:-1]])
    flat = (offs[:, None] + krig_idx).reshape(-1)
    smp = y[flat]
    h = np.maximum(smp.astype(np.float32) @ W1 + b1, 0.0)
    logits = h @ W2 + b2
    z = logits + gumbel
    ind = (z[:, 1] > z[:, 0]).astype(np.float32)
    val = ind[:, None] * mask * smp
    out = x.copy()
    out[flat] = val
    return out


def kernel(**inputs) -> np.ndarray:
    x = np.ascontiguousarray(inputs["x"], dtype=np.float32)
    y = np.ascontiguousarray(inputs["y"], dtype=np.float32)
    W1 = np.ascontiguousarray(inputs["W1"], dtype=np.float32)
    b1 = np.ascontiguousarray(inputs["b1"], dtype=np.float32)
    W2 = np.ascontiguousarray(inputs["W2"], dtype=np.float32)
    b2 = np.ascontiguousarray(inputs["b2"], dtype=np.float32)
    mask = np.ascontiguousarray(inputs["mask"], dtype=np.float32)
    gumbel = np.ascontiguousarray(inputs["gumbel"], dtype=np.float32)
    krig = np.asarray(inputs["krig_idx"]).astype(np.int64)
    ion = np.asarray(inputs["idx_of_node"]).astype(np.int64)

    if (
        x.shape != (BS * N, S)
        or krig.shape != (BS, K)
        or not np.all(ion == N)
        or krig.min() < 0
        or krig.max() >= N
    ):
        return _numpy_fallback(
            x, y, W1, b1, W2, b2, mask, gumbel,
            np.asarray(inputs["krig_idx"]), ion,
        )

    from concourse.bass_utils import run_bass_kernel_spmd

    nc = _get_nc()

    # Host layout prep (sharding/marshalling only).
    flat_all = ((np.arange(BS, dtype=np.int64) * N)[:, None] + krig).reshape(-1)
    xz = x.copy()
    xz[flat_all] = 0.0  # scatter targets become add-onto-zero
    yp = np.zeros((M, R, SP), dtype=np.float32)
    yp[:, :, :S] = y.reshape(M, R, S)
    mask_r = np.ascontiguousarray(
        mask.reshape(M, T, P, S).transpose(0, 2, 1, 3)
    ).reshape(M, P, T * S)
    gum_r = np.ascontiguousarray(
        gumbel.reshape(M, T, P, AUG).transpose(0, 2, 1, 3)
    ).reshape(M, P, T * AUG)
    b2_r = np.ascontiguousarray(np.broadcast_to(b2.reshape(1, AUG), (P, AUG)))

    def wrap16(stream):
        # device consumes index i at idxs[i % 16, i // 16], replicated x8
        return np.ascontiguousarray(
            np.tile(stream.reshape(-1, 16).T.astype(np.int16), (M, 1))
        )

    in_maps = []
    for m in range(M):
        fl = flat_all[m * J : (m + 1) * J] - m * R  # [J] core-local rows
        sidx_stream = fl - (np.arange(J, dtype=np.int64) // JC) * RC
        in_maps.append(
            {
                "x": xz[m * R : (m + 1) * R],
                "yp": yp[m],
                "mask": mask_r[m],
                "gum": gum_r[m],
                "W1": W1,
                "b1": b1.reshape(HID, 1),
                "W2": W2,
                "b2": b2_r,
                "ident": np.eye(P, dtype=np.float32),
                "gidx": wrap16(fl),
                "sidx": wrap16(sidx_stream),
            }
        )

    import os

    trace = bool(int(os.environ.get("KERNEL_TRACE", "0")))
    res = run_bass_kernel_spmd(nc, in_maps, core_ids=list(range(M)), trace=trace)
    _cache["last_res"] = res

    out = np.empty((BS * N, S), dtype=np.float32)
    for m in range(M):
        for g in range(C):
            out[m * R + g * RC : m * R + (g + 1) * RC] = res.results[m][f"out{g}"][
                :, :S
            ]
    return out
